# revision 49
# baseline (speedup 1.0000x reference)
"""Trainium2 Bass kernel for the DLI (dialogue-turn ordering) loss — v2.

Math (exact reduction of the reference):
  With 2 classes, NLL(label y) = softplus(l_{1-y} - l_y).
  u[b,j] = enc[b,j] @ (W[:D,1]-W[:D,0]),
  v[b,k] = enc[b,k] @ (W[D:,1]-W[D:,0]),
  c      = b[1]-b[0],  d[b,j,k] = u[b,j] + v[b,k] + c
  label = 1 iff k == j-1; valid pairs: k < j < len_b;  softplus(-d) = softplus(d) - d
  =>  sum_nll = sum_{valid} softplus(d) - sum_{b, 1<=j<len_b} d[b,j,j-1]
  loss = sum_nll / max(n_valid, 1)

v2 strategy (ragged packing):
  Only rows j < len_b matter.  The host sorts the 64 batches by length,
  deals rank-groups of 8 across the 8 cores (slot s holds the 8 batches of
  rank 8s..8s+7, one per core; its row count R_s = the group max), and
  first-fit-decreasing packs the 8 slots into bins of <=128 rows.  Each
  core ships only the packed valid rows (~560 of 1024), so HBM traffic and
  the number of [128, 2048] dot passes both drop ~2x vs dense batching.

  The kernel program is specialized on the slot/bin structure (recompiled
  if the mask signature changes; cached otherwise).

Engine split per core (all bins share one SPMD program):
  DMA:    per-bin SWDGE casting DMAs (HBM f32 -> SBUF bf16); w rows arrive
          pre-replicated 16x from the host and broadcast 8x by HWDGE;
          consts (identity / triangular NEG mask / ones / row masks) in one
          HWDGE aux load
  DVE:    bf16 2x tensor_tensor multiplies for every dot, fold-1024 adds
          for some dots (shifts reduce work DVE-ward), SU bias adds, vt
          copies, final reductions, memsets
  ACT:    dot reductions (Copy + accum_out), per-pair Exp (bias = u+rowmask,
          reads PSUM) and Ln(x+1) with fused row-sum
  PE:     additive lower-triangular NEG mask + v-row broadcast accumulated
          into PSUM (kills the old DVE tri01 multiply), v-column transposes
  GpSimd: SWDGE DMA triggers ONLY (any ucode op would force a Q7 dge-drain
          against the in-flight casting DMAs and stall for their duration)
All ACT functions (Copy/Exp/Ln) are forced into the single
natural_log_exp_and_others table so the act-table is loaded exactly once.
"""

import glob
import json
import os
import shutil
import sys
import tempfile

if "/opt/trn_rl_repo" not in sys.path:
    sys.path.insert(0, "/opt/trn_rl_repo")


def _force_combined_act_table():
    """Point walrus at an act_info.json holding only natural_log_exp_and_others
    (contains exp+ln+copy), so every ACTIVATE shares one table."""
    if os.environ.get("BASS_ACT_ROOT_JSON_PATH"):
        return
    from neuronxcc.driver.Job import Job  # type: ignore

    pwp = None
    for cand in glob.glob(os.path.join(Job.getPackageDir(), "pwp", "pwp_bin_*")):
        if os.path.exists(os.path.join(cand, "act_info.json")):
            pwp = cand
            break
    if pwp is None:
        return
    info = json.load(open(os.path.join(pwp, "act_info.json")))
    keep = [t for t in info.get("act_func_sets", [])
            if t.get("name") == "natural_log_exp_and_others"]
    if not keep:
        return
    out_dir = os.path.join(tempfile.gettempdir(), "dli_act_combined")
    os.makedirs(out_dir, exist_ok=True)
    for t in keep:
        for k in info.get("pwp_file_keys", []):
            f = t.get(k)
            src = os.path.join(pwp, f) if f else None
            if src and os.path.exists(src):
                dst = os.path.join(out_dir, f)
                if not os.path.exists(dst):
                    shutil.copy(src, dst)
    info = dict(info)
    info["act_func_sets"] = keep
    with open(os.path.join(out_dir, "act_info.json"), "w") as f:
        json.dump(info, f)
    os.environ["BASS_ACT_ROOT_JSON_PATH"] = os.path.join(out_dir, "act_info.json")


_force_combined_act_table()

from contextlib import ExitStack

import ml_dtypes
import numpy as np

import concourse.bacc as bacc
import concourse.bass as bass
import concourse.hw_specs as hw_specs
import concourse.mybir as mybir
import concourse.tile as tile

# Make bass's act-table placement agree with the trimmed act_info.json walrus
# sees: only the combined exp+ln+copy table exists, so every ACTIVATE maps to
# act_func_set_id 0 and the table is loaded exactly once.
_orig_get_act_tables = hw_specs.get_activation_tables


def _combined_act_tables(module_arch):
    tabs = _orig_get_act_tables(module_arch)
    kept = {k: v for k, v in tabs.items() if k == "natural_log_exp_and_others"}
    return kept if kept and os.environ.get("BASS_ACT_ROOT_JSON_PATH") else tabs


hw_specs.get_activation_tables = _combined_act_tables
bacc.get_activation_tables = _combined_act_tables

# Cheaper kernel teardown: drain + one all-engine barrier + sem clear. The
# stock epilogue adds a second all-engine barrier after the clear; engines
# that pass the first barrier only run their terminal branch, and the next
# execution starts only after every engine (incl. the clearing one) halts,
# so the second barrier only adds ~4us of EVSEM latency.
from concourse.vector_clock import ScopedClock as _ScopedClock


def _cheap_drain_and_barrier(self, tick_clock, wait_clock):
    drain_inst = self.nc.sync.drain()
    wait_clock.add_sem_waits(
        drain_inst.ins, _ScopedClock({None: tick_clock.global_clock})
    )
    self.nc.all_engine_barrier()
    popped = self.nc._tile_sem_poison_stack.pop()
    assert popped is self._sem_poison
    self.nc.clear_and_free_semaphores(list(self.sems.allocated().values()))


tile.TileContext._drain_and_barrier = _cheap_drain_and_barrier

# Slimmer kernel semaphore universe: the epilogue resets every sem in
# get_kernel_semaphore_range() one at a time (~54ns each on GpSimd). This
# kernel uses ~25 sems, so cap the range instead of sweeping all 150.
_orig_sem_range = bass.get_kernel_semaphore_range


def _slim_sem_range():
    r = _orig_sem_range()
    return range(r.start, min(r.stop, r.start + 44))


bass.get_kernel_semaphore_range = _slim_sem_range

# Skip the trailing "safety" barrier in Bass.reset (the kernel epilogue):
# after the per-engine sem sweeps, engines just halt; NRT waits for all
# engines to halt before the next execution, and the next execution opens
# with its own all-engine barrier. Saves the ~2us chained-EVSEM latency.
_orig_reset = bass.Bass.reset


def _fast_reset(self, *args, **kwargs):
    orig_bar = bass.Bass.all_engine_barrier
    calls = [0]

    def _bar(s, **kw):
        calls[0] += 1
        if calls[0] >= 2:
            return None
        return orig_bar(s, **kw)

    bass.Bass.all_engine_barrier = _bar
    try:
        return _orig_reset(self, *args, **kwargs)
    finally:
        bass.Bass.all_engine_barrier = orig_bar


bass.Bass.reset = _fast_reset

F32 = mybir.dt.float32
BF16 = mybir.dt.bfloat16
ALU = mybir.AluOpType
ACTF = mybir.ActivationFunctionType
AXX = mybir.AxisListType.X

BSZ, L, D = 64, 128, 2048
N_CORES = 8
NB = BSZ // N_CORES  # batch slots per core
NEG = -30000.0  # additive mask value; exp(NEG + anything finite) == 0 in f32


def _plan(lens):
    """Shared (cross-core) packing plan from the 64 lengths.

    Returns dict with:
      order   [64] batch indices sorted by len desc
      R       [NB] padded row count per slot (max over its rank group)
      bins    list of list of slot ids (first-fit decreasing, cap 128)
      off     [NB] partition offset of each slot inside its bin
      fill    per-bin total rows
      row0    per-bin starting row in the packed enc tensor
      sched   bin visit order (largest fill first)
    """
    lens = np.asarray(lens, dtype=np.int64)
    order = np.argsort(-lens, kind="stable")
    R = [int(lens[order[NB * s]]) for s in range(NB)]
    bins, binfill = [], []
    off = [0] * NB
    for s in sorted(range(NB), key=lambda s: -R[s]):
        for t, b in enumerate(bins):
            if binfill[t] + R[s] <= L:
                off[s] = binfill[t]
                b.append(s)
                binfill[t] += R[s]
                break
        else:
            bins.append([s])
            binfill.append(R[s])
            off[s] = 0
    sched = sorted(range(len(bins)), key=lambda t: -binfill[t])
    # group consecutive sched bins into shared DMA triggers (equal padded
    # fill inside a group; few triggers so they fit one GpSimd block)
    # full-128-partition DMAs only: non-128 partition counts collapse the
    # 16-engine descriptor spray onto one SDMA engine (observed ~13 GB/s)
    # first group is a single bin so the first dots start as early as
    # possible; later bins pair up to keep trigger count low
    groups = []
    for t in sched:
        if groups and len(groups[-1]["bins"]) < 2 and len(groups) > 1:
            groups[-1]["bins"].append(t)
        else:
            groups.append({"bins": [t], "F": L})
    acc = 0
    row0 = {}
    for g in groups:
        g["row0"] = acc
        for i, t in enumerate(g["bins"]):
            row0[t] = acc + i * g["F"]
        acc += len(g["bins"]) * g["F"]
    return dict(order=order, R=R, bins=bins, off=off, fill=binfill,
                row0=row0, tot=acc, sched=sched, groups=groups)


def build_program(plan):
    R, bins, off = plan["R"], plan["bins"], plan["off"]
    fill, row0, sched = plan["fill"], plan["row0"], plan["sched"]
    nbins = len(bins)
    TOT = plan["tot"]
    slot_bin = {s: t for t, b in enumerate(bins) for s in b}

    # aux layout (f32, [L, AUXW]):
    #   identity | per-bin TRI masks | ones col | rmMp | rm12p
    IDC = 0
    TRC = L                       # nbins blocks of [L, L]
    ONC = TRC + nbins * L         # single all-ones column
    RMC = ONC + 1
    R12C = RMC + nbins
    AUXW = R12C + 2 * nbins

    # dots fold 1024 first (shifts ~0.86us/dot of reduce work ACT->DVE) —
    # except the first bin's: ACT is idle early, so full-width reduces there
    # fill its gap while relieving the busier DVE
    fold_set = {(t, cls) for t in sched[1:] for cls in (1, 0)}



    nc = bacc.Bacc("TRN2", target_bir_lowering=False, debug=False, num_devices=1)

    enc = nc.dram_tensor("enc", [TOT, D], F32, kind="ExternalInput").ap()
    wuv = nc.dram_tensor("wuv", [2, L, D], BF16, kind="ExternalInput").ap()
    aux = nc.dram_tensor("aux", [L, AUXW], F32, kind="ExternalInput").ap()
    selr = nc.dram_tensor("selr", [1, NB * L], F32, kind="ExternalInput").ap()
    out = nc.dram_tensor("out", [1, 1], F32, kind="ExternalOutput").ap()

    with tile.TileContext(nc) as tc, ExitStack() as ctx:
        consts = ctx.enter_context(tc.tile_pool(name="consts", bufs=1))
        accs = ctx.enter_context(tc.tile_pool(name="accs", bufs=1))
        enc_pool = ctx.enter_context(tc.tile_pool(name="enc", bufs=1))
        prod_pool = ctx.enter_context(tc.tile_pool(name="prod", bufs=3))
        fold_pool = ctx.enter_context(tc.tile_pool(name="fold", bufs=2))
        ex_pool = ctx.enter_context(tc.tile_pool(name="ex", bufs=3))
        ln_pool = ctx.enter_context(tc.tile_pool(name="ln", bufs=2))
        psd_pool = ctx.enter_context(tc.tile_pool(name="psd", bufs=3, space="PSUM"))
        psv_pool = ctx.enter_context(tc.tile_pool(name="psv", bufs=2, space="PSUM"))
        psm_pool = ctx.enter_context(tc.tile_pool(name="psm", bufs=1, space="PSUM"))

        # ---- input DMAs (aux/sel issued after the w halves below) ----
        aux_sb = consts.tile([L, AUXW], F32)
        sel_sb = consts.tile([1, NB * L], F32)
        ident = aux_sb[:, IDC:IDC + L]
        tri = [aux_sb[:, TRC + t * L:TRC + (t + 1) * L] for t in range(nbins)]
        ones_col = aux_sb[:, ONC:ONC + 1]
        rmMp = aux_sb[:, RMC:RMC + nbins]
        rm12p = aux_sb[:, R12C:R12C + 2 * nbins]

        # w rows in halves, wv first: the first (halved) muls are v-dots and
        # need only wv's low half
        wu_b = consts.tile([L, D], BF16)
        wv_b = consts.tile([L, D], BF16)
        H = D // 2
        # wv on the sync HWDGE ring, wu on the scalar ring — the two rings
        # drain in parallel (the Scalar teardown sem sweep is unconditional,
        # so using its ring costs nothing extra)
        nc.sync.dma_start(wv_b[:, 0:H], wuv[1, :, 0:H])
        nc.scalar.dma_start(wu_b[:, 0:H], wuv[0, :, 0:H])
        nc.sync.dma_start(wv_b[:, H:D], wuv[1, :, H:D])
        nc.scalar.dma_start(wu_b[:, H:D], wuv[0, :, H:D])
        nc.sync.dma_start(aux_sb[:], aux[:])
        nc.sync.dma_start(sel_sb[:], selr[:])

        E = {}
        halved = set()
        for gi, g in enumerate(plan["groups"]):
            n, Fg, r0 = len(g["bins"]), g["F"], g["row0"]
            gt = enc_pool.tile([L, n * D], BF16, tag=f"enc{gi}", name=f"encg{gi}")
            if True:
                # all groups arrive in D-halves so the muls (and the ACT
                # reduce pipeline behind them) start earlier
                for h in range(2):
                    lo, hi = h * (D // 2), (h + 1) * (D // 2)
                    nc.gpsimd.dma_start(
                        gt[:].rearrange("p (i d) -> p i d", i=n)[:, :, lo:hi],
                        enc[r0:r0 + n * Fg, lo:hi].rearrange(
                            "(i p) d -> p i d", i=n),
                    )
                halved.update(g["bins"])
            elif n == 1:
                nc.gpsimd.dma_start(gt[:], enc[r0:r0 + Fg, :])
            else:
                nc.gpsimd.dma_start(
                    gt[:].rearrange("p (i d) -> p i d", i=n),
                    enc[r0:r0 + n * Fg, :].rearrange("(i p) d -> p i d", i=n),
                )
            for i, t in enumerate(g["bins"]):
                E[t] = gt[:, i * D:(i + 1) * D]

        # ---- accumulators ----
        UV = accs.tile([L, 2 * nbins], F32)  # cols [0,nbins)=u per bin, [nbins,2n)=v
        nc.vector.memset(UV[:], 0.0)
        SU = accs.tile([L, nbins], F32)
        RS = accs.tile([L, nbins], F32)
        nc.vector.memset(RS[:], 0.0)
        vt_sb = [accs.tile([1, L], F32, name=f"vt{t}") for t in range(nbins)]

        # ---- dots ----
        def dot(t, cls):
            F = fill[t]
            w_tile = wv_b if cls == 1 else wu_b
            col = (nbins if cls == 1 else 0) + t
            prod = prod_pool.tile([L, D], BF16, tag="p", name=f"p{t}_{cls}")
            if t in halved:
                for h in range(2):
                    lo, hi = h * (D // 2), (h + 1) * (D // 2)
                    nc.vector.tensor_mul(prod[0:F, lo:hi], E[t][0:F, lo:hi],
                                         w_tile[0:F, lo:hi])
            else:
                nc.vector.tensor_mul(prod[0:F, :], E[t][0:F, :], w_tile[0:F, :])
            if (t, cls) in fold_set:
                wid = D // 2
                p2 = fold_pool.tile([L, wid], BF16, tag="f", name=f"f{t}_{cls}")
                nc.vector.tensor_add(p2[0:F, :], prod[0:F, 0:wid],
                                     prod[0:F, wid:D])
                red_src = p2[0:F, :]
            else:
                wid = D
                red_src = prod[0:F, :]
            junk = prod_pool.tile([L, D], BF16, tag="junk", bufs=2,
                                  name=f"j{t}_{cls}")
            nc.scalar.activation(junk[0:F, 0:wid], red_src, ACTF.Copy,
                                 accum_out=UV[0:F, col:col + 1])

        # ---- phase B per bin (packed partition space, all base-0) ----
        def phase_b(t):
            F = fill[t]
            psd = psd_pool.tile([L, L], F32, tag="d", name=f"d{t}")
            # start with the per-bin triangular/validity NEG mask
            # (k columns >= F belong to no slot and always mask to zero)
            nc.tensor.matmul(psd[0:F, 0:F], lhsT=ident[0:F, 0:F],
                             rhs=tri[t][0:F, 0:F], start=True, stop=False)
            # + per-slot v rows via selector columns
            for i, s in enumerate(bins[t]):
                o, r = off[s], R[s]
                nc.tensor.matmul(psd[0:F, o:o + r],
                                 lhsT=sel_sb[0:1, s * L:s * L + F],
                                 rhs=vt_sb[t][0:1, o:o + r],
                                 start=False, stop=(i == len(bins[t]) - 1))
            ex = ex_pool.tile([L, L], F32, tag="e", name=f"e{t}")
            nc.scalar.activation(ex[0:F, 0:F], psd[0:F, 0:F], ACTF.Exp,
                                 bias=SU[0:F, t:t + 1])
            lnt = ln_pool.tile([L, L], F32, tag="l", name=f"ln{t}")
            nc.scalar.activation(lnt[0:F, 0:F], ex[0:F, 0:F], ACTF.Ln, bias=1.0,
                                 accum_out=RS[0:F, t:t + 1])

        for t in sched:
            dot(t, 1)   # v first: gates the vt transpose
            psv = psv_pool.tile([1, L], F32, tag="v", name=f"v{t}")
            nc.tensor.matmul(psv[:], lhsT=UV[:, nbins + t:nbins + t + 1],
                             rhs=ident[:, :], is_transpose=True)
            nc.vector.tensor_copy(vt_sb[t][:], psv[:])
            dot(t, 0)
            nc.vector.tensor_add(SU[0:fill[t], t:t + 1], UV[0:fill[t], t:t + 1],
                                 rmMp[0:fill[t], t:t + 1])
            phase_b(t)

        # ---- diagonal (label-1) terms + final reduction ----
        dUV = accs.tile([L, 2 * nbins], F32)
        nc.vector.tensor_mul(dUV[:], UV[:], rm12p[:])
        dr = accs.tile([L, 1], F32)
        nc.vector.reduce_sum(dr[:], dUV[:], axis=AXX)
        accA = accs.tile([L, 1], F32)
        nc.vector.reduce_sum(accA[:], RS[:], axis=AXX)
        nc.vector.tensor_sub(accA[:], accA[:], dr[:])
        psum_s = psm_pool.tile([1, 1], F32, tag="ps")
        nc.tensor.matmul(psum_s[:], lhsT=accA[:], rhs=ones_col[:])
        out_t = accs.tile([1, 1], F32)
        nc.vector.tensor_copy(out_t[:], psum_s[:])
        nc.sync.dma_start(out[:], out_t[:])

    nc.compile()
    return nc


_NC_CACHE = {}


def _get_nc(plan):
    key = tuple(plan["R"])
    if key not in _NC_CACHE:
        _NC_CACHE[key] = build_program(plan)
    return _NC_CACHE[key]


def _prep(encoder_output, mask, W, b):
    """Host-side prep: pack ragged rows, build derived small tensors."""
    enc = np.asarray(encoder_output, dtype=np.float32)
    W = np.asarray(W, dtype=np.float32)
    b = np.asarray(b, dtype=np.float32).reshape(2)
    mask = np.asarray(mask)
    c = float(b[1] - b[0])
    lens = mask.astype(np.int64).sum(axis=1)  # [BSZ]
    plan = _plan(lens)
    order, R, bins, off = plan["order"], plan["R"], plan["bins"], plan["off"]
    fill, row0, TOT = plan["fill"], plan["row0"], plan["tot"]
    nbins = len(bins)
    slot_bin = {s: t for t, b_ in enumerate(bins) for s in b_}

    wuv = np.stack([W[:D, 1] - W[:D, 0], W[D:, 1] - W[D:, 0]])
    wuv16 = np.broadcast_to(
        wuv.astype(ml_dtypes.bfloat16)[:, None, :], (2, L, D)
    ).copy()

    IDC = 0
    TRC = L
    ONC = TRC + nbins * L
    RMC = ONC + 1
    R12C = RMC + nbins
    AUXW = R12C + 2 * nbins
    aux_base = np.zeros((L, AUXW), dtype=np.float32)
    aux_base[:, IDC:IDC + L] = np.eye(L, dtype=np.float32)
    aux_base[:, ONC] = 1.0
    # per-bin diagonal-block triangular masks: for slot s at offset o,
    # tri[o+j, o+k] = 0 if k < j else NEG; everything else NEG
    aux_base[:, TRC:TRC + nbins * L] = NEG
    selrow = np.zeros((1, NB * L), dtype=np.float32)
    for s in range(NB):
        t = slot_bin[s]
        o, r = off[s], R[s]
        jl = np.arange(r)
        blk = np.where(jl[None, :] < jl[:, None], 0.0, NEG)  # [r, r], k<j -> 0
        aux_base[o:o + r, TRC + t * L + o:TRC + t * L + o + r] = blk
        selrow[0, s * L + o:s * L + o + r] = 1.0

    maps = []
    for cid in range(N_CORES):
        batches = {s: int(order[NB * s + cid]) for s in range(NB)}
        enc_send = np.empty((TOT, D), dtype=np.float32)
        auxc = aux_base.copy()
        auxc[:, RMC:RMC + nbins] = NEG + c
        enc_send[:] = 0.0
        for s in range(NB):
            t = slot_bin[s]
            o, r = off[s], R[s]
            bb = batches[s]
            ln_b = int(lens[bb])
            enc_send[row0[t] + o:row0[t] + o + r] = enc[bb, 0:r]
            jl = np.arange(r)
            auxc[o:o + r, RMC + t] = np.where(jl < ln_b, 0.0, NEG) + c
            auxc[o:o + r, R12C + t] = ((jl >= 1) & (jl < ln_b)).astype(np.float32)
            auxc[o:o + r, R12C + nbins + t] = (jl < ln_b - 1).astype(np.float32)
        maps.append({"enc": enc_send, "wuv": wuv16, "aux": auxc,
                     "selr": selrow})

    diag_c = float(c * (lens - 1).sum())
    n_valid = int((lens * (lens - 1) // 2).sum())
    return plan, maps, diag_c, n_valid


def kernel(encoder_output, mask, W, b, _run_kwargs=None):
    from concourse.bass_utils import run_bass_kernel_spmd

    plan, maps, diag_c, n_valid = _prep(encoder_output, mask, W, b)
    nc = _get_nc(plan)
    res = run_bass_kernel_spmd(nc, maps, core_ids=list(range(N_CORES)),
                               **(_run_kwargs or {}))
    total = float(sum(np.float64(r["out"][0, 0]) for r in res.results))
    total -= diag_c
    loss = total / max(n_valid, 1)
    out = np.array(loss, dtype=np.float32)
    if _run_kwargs is not None:
        return out, res
    return out
